# revision 2
# baseline (speedup 1.0000x reference)
"""Trainium2 Bass kernel for pre-LN multi-head self-attention.

One batch element per core (8 cores, data parallel). Host fuses weights:
M_h = Wq_h^T Wk_h (scores become xn M xn^T) and VO_h = W_o_h W_v_h (out-proj
folded into V). Per core:

  Phase 1 (interleaved): LN -> xnT fp16; V_h = xn VO_h^T -> vt[h] bf16;
  G^T_h = M_h^T xn^T -> gT[h] fp16 for ALL heads.

  Phase 2 (v20): 32 stages t = ib*8 + h with a one-head software pipeline.
  Stage t: scores(h, ib) fp16 -> f32 PSUM; exp (scalar, constant -75 shift)
  -> e bf16; DVE accumulates the 8 e-tiles (bf16), folds, gpsimd
  partition_all_reduce gives the per-query softmax row-sum broadcast to all
  partitions; reciprocal -> rinv bf16; e *= rinv in place (pre-normalized
  attention weights). AV matmuls of stage t-1 interleave with stage t's
  scores chunk-by-chunk and accumulate ALL HEADS into a single PSUM tile
  po[i, d] (start at h==0, stop at h==7) -- no per-head evacuate/normalize/
  add. Per ib: one DVE evacuation + DMA straight out.

dtypes: fp16 scores side (bf16 logit noise costs 1.4e-2 rel err; fp16
2.4e-3), bf16 exp/V side (exp needs bf16 exponent range). Row-sums
accumulate bf16 pairwise + f32 partition reduce; scale error ~2^-9 per
query is within budget.
"""

import numpy as np

import concourse.bass as bass
import concourse.bass_isa as bass_isa
import concourse.mybir as mybir
import concourse.tile as tile
from concourse import bacc
from concourse.bass_utils import run_bass_kernel_spmd

F32 = mybir.dt.float32
BF16 = mybir.dt.bfloat16
F16 = mybir.dt.float16

N_CORES = 8
N = 2048
DIM = 256
H = 8
EXP_SHIFT = 75.0

NT = N // 128
DC = DIM // 128
IB = N // 512
JP = NT // 2
VW = 256


def build_nc_v20():
    nc = bacc.Bacc("TRN2", target_bir_lowering=False, debug=False,
                   num_devices=N_CORES)
    x_d = nc.dram_tensor("x", [N, DIM], F32, kind="ExternalInput")
    wq_d = nc.dram_tensor("wqkvT", [DIM, 2 * H * DIM], F16, kind="ExternalInput")
    id_d = nc.dram_tensor("ident", [128, 128], F32, kind="ExternalInput")
    out_d = nc.dram_tensor("out", [N, DIM], F32, kind="ExternalOutput")

    with tile.TileContext(nc) as tc:
        with (
            tc.tile_pool(name="singles", bufs=1) as singles,
            tc.tile_pool(name="xin", bufs=4) as xin,
            tc.tile_pool(name="lnst", bufs=6) as lnst,
            tc.tile_pool(name="etp", bufs=12) as etp,
            tc.tile_pool(name="accp", bufs=2) as accp,
            tc.tile_pool(name="acc2p", bufs=2) as acc2p,
            tc.tile_pool(name="raccp", bufs=2) as raccp,
            tc.tile_pool(name="rinvp", bufs=2) as rinvp,
            tc.tile_pool(name="ocp", bufs=1) as ocp,
            tc.tile_pool(name="ps_sc", bufs=2, space="PSUM") as ps_sc,
            tc.tile_pool(name="ps_acc", bufs=1, space="PSUM") as ps_acc,
        ):
            ident = singles.tile([128, 128], F32, tag="ident")
            nc.sync.dma_start(ident[:], id_d.ap()[:, :])
            eps_t = singles.tile([128, 1], F32, tag="eps")
            nc.vector.memset(eps_t, 1e-5)
            shift_t = singles.tile([128, 1], F32, tag="shift")
            nc.vector.memset(shift_t, -EXP_SHIFT)

            wqs = [[singles.tile([128, 2048], F16, tag=f"wq{dc}_{s}",
                                 name=f"wq{dc}_{s}") for s in range(2)]
                   for dc in range(DC)]
            xnT = [singles.tile([128, N], F16, tag=f"xnT{dc}", name=f"xnT{dc}")
                   for dc in range(DC)]
            vt = [singles.tile([128, NT, VW], BF16, tag=f"vt{h}", name=f"vt{h}")
                  for h in range(H)]
            gTa = [singles.tile([128, DC, N], F16, tag=f"gT{h}", name=f"gT{h}")
                   for h in range(H)]

            def emit_ln(tcn):
                xt = xin.tile([128, DIM], F32, tag="xt")
                nc.sync.dma_start(xt[:], x_d.ap()[tcn * 128:(tcn + 1) * 128, :])
                stats = lnst.tile([128, 6], F32, tag="stats")
                nc.vector.bn_stats(out=stats[:], in_=xt[:])
                mv = lnst.tile([128, 2], F32, tag="mv")
                nc.vector.bn_aggr(out=mv[:], in_=stats[:])
                nc.scalar.activation(
                    out=mv[:, 1:2], in_=mv[:, 1:2],
                    func=mybir.ActivationFunctionType.Sqrt,
                    bias=eps_t[:, 0:1], scale=1.0)
                nc.vector.reciprocal(out=mv[:, 1:2], in_=mv[:, 1:2])
                # single-op normalize: the shortest cross-engine chain wins
                # over engine balance here (phase 1 is latency-bound).
                nc.vector.tensor_scalar(
                    out=xt[:], in0=xt[:], scalar1=mv[:, 0:1], scalar2=mv[:, 1:2],
                    op0=mybir.AluOpType.subtract, op1=mybir.AluOpType.mult)
                for dc in range(DC):
                    pst = ps_sc.tile([128, 2, 512], F32, tag="sc", name="pst")
                    nc.tensor.transpose(
                        pst[:, 0, :128], xt[:, dc * 128:(dc + 1) * 128], ident[:])
                    nc.vector.tensor_copy(
                        out=xnT[dc][:, tcn * 128:(tcn + 1) * 128],
                        in_=pst[:, 0, :128])

            vcopy_idx = [0]

            def emit_vbuild(tc2):
                # V-build PSUMs live in the acc pool (its 4 banks are idle
                # until attention starts), taking pressure off the 2 "sc"
                # slots that pace the LN-transpose/G-build rotation.
                for g in range(2):  # head-pair groups (2 pairs per acc tile)
                    ps = ps_acc.tile([128, 4, 512], F32, tag="acc", name="psv")
                    for pi in range(2):
                        p = 2 * g + pi
                        for half in range(2):
                            tcn = tc2 * 2 + half
                            for dc in range(DC):
                                nc.tensor.matmul(
                                    ps[:, 2 * pi + half, :],
                                    xnT[dc][:, tcn * 128:(tcn + 1) * 128],
                                    wqs[dc][1][:, p * 512:(p + 1) * 512],
                                    start=(dc == 0), stop=(dc == DC - 1))
                        for s in range(2):
                            # V copies split ~2:1 scalar:vector so neither
                            # queue paces phase 1.
                            dst = vt[2 * p + s][:, tc2 * 2:tc2 * 2 + 2, 0:256]
                            src = ps[:, 2 * pi:2 * pi + 2,
                                     s * 256:(s + 1) * 256]
                            if vcopy_idx[0] % 3 != 2:
                                nc.scalar.copy(out=dst, in_=src)
                            else:
                                nc.vector.tensor_copy(out=dst, in_=src)
                            vcopy_idx[0] += 1

            def emit_gbuild_half(h, ib2):
                gT = gTa[h]
                for mc in range(DC):
                    ps = ps_sc.tile([128, 2, 512], F32, tag="sc", name="psg")
                    for half in range(2):
                        jb = ib2 * 2 + half
                        for dc in range(DC):
                            nc.tensor.matmul(
                                ps[:, half, :],
                                wqs[dc][0][:, h * DIM + mc * 128:
                                           h * DIM + (mc + 1) * 128],
                                xnT[dc][:, jb * 512:(jb + 1) * 512],
                                start=(dc == 0), stop=(dc == DC - 1))
                    dst = gT[:, mc, ib2 * 1024:(ib2 + 1) * 1024]
                    # second halves run concurrently with phase-2 exp: keep
                    # them off the scalar engine (activation-table swaps).
                    if ib2 == 0 and (h + mc) % 2 == 0:
                        nc.scalar.copy(out=dst, in_=ps[:, :, :])
                    else:
                        nc.vector.tensor_copy(out=dst, in_=ps[:, :, :])

            emit_ln(0)
            emit_ln(1)
            for dc in range(DC):  # VO weights behind x0/x1 (V build first)
                nc.sync.dma_start(
                    wqs[dc][1][:],
                    wq_d.ap()[dc * 128:(dc + 1) * 128, 2048:4096])
            for tc2 in range(1, NT // 2):
                emit_ln(tc2 * 2)
                if tc2 == 1:
                    for dc in range(DC):  # M weights behind x2
                        nc.sync.dma_start(
                            wqs[dc][0][:],
                            wq_d.ap()[dc * 128:(dc + 1) * 128, 0:2048])
                emit_ln(tc2 * 2 + 1)
                emit_vbuild(tc2 - 1)
                if tc2 >= 4:  # G first halves once token chunks 0..7 exist
                    emit_gbuild_half(2 * (tc2 - 4), 0)
                    emit_gbuild_half(2 * (tc2 - 4) + 1, 0)
            emit_vbuild(NT // 2 - 1)

            # ---- Phase 2: pipelined scores/softmax (stage t) + AV (t-1) ----
            po_cur = [None]

            def emit_av_pair(es_prev, h_prev, jp):
                if h_prev == 0 and jp == 0:
                    po_cur[0] = ps_acc.tile([128, 4, 512], F32, tag="acc",
                                            name="po")
                po = po_cur[0]
                for half in range(2):
                    jc = jp * 2 + half
                    for k in range(4):
                        nc.tensor.matmul(
                            po[:, k, 0:256],
                            es_prev[jp][:, half, k * 128:(k + 1) * 128],
                            vt[h_prev][:, jc, 0:256],
                            start=(h_prev == 0 and jc == 0),
                            stop=(h_prev == H - 1 and jc == NT - 1))

            def emit_evac(ib):
                po = po_cur[0]
                oc = ocp.tile([128, 4, 256], F32, tag="oc")
                nc.vector.tensor_copy(out=oc[:, 0:2, :], in_=po[:, 0:2, 0:256])
                nc.vector.tensor_copy(out=oc[:, 2:4, :], in_=po[:, 2:4, 0:256])
                for k in range(4):
                    i128 = ib * 4 + k
                    nc.sync.dma_start(
                        out_d.ap()[i128 * 128:(i128 + 1) * 128, :],
                        oc[:, k, :])

            def emit_stage(t, av_prev):
                ib, h = divmod(t, H)
                es = []
                acc = accp.tile([128, 2, 512], BF16, tag="acc")
                for jp in range(JP):
                    ps = ps_sc.tile([128, 2, 512], F32, tag="sc", name="pss")
                    for half in range(2):
                        jc = jp * 2 + half
                        for dc in range(DC):
                            nc.tensor.matmul(
                                ps[:, half, :],
                                gTa[h][:, dc, jc * 128:(jc + 1) * 128],
                                xnT[dc][:, ib * 512:(ib + 1) * 512],
                                start=(dc == 0), stop=(dc == DC - 1))
                    e_t = etp.tile([128, 2, 512], BF16, tag="et")
                    nc.scalar.activation(
                        out=e_t[:, :, :], in_=ps[:, :, :],
                        func=mybir.ActivationFunctionType.Exp,
                        bias=shift_t[:, 0:1], scale=1.0)
                    es.append(e_t)
                    # row-sum accumulation chain on DVE (bf16: only the
                    # per-query softmax scale is affected, ~2^-9).
                    with nc.allow_low_precision(reason="softmax rowsum scale"):
                        if jp == 1:
                            nc.vector.tensor_add(
                                out=acc[:], in0=es[0][:], in1=es[1][:])
                        elif jp > 1:
                            nc.vector.tensor_add(
                                out=acc[:], in0=acc[:], in1=e_t[:])
                    if av_prev is not None:
                        emit_av_pair(av_prev[0], av_prev[1], jp)
                # evacuate the finished po (all 8 heads) before the DVE gets
                # busy with this stage's normalize; frees the bank for the
                # next ib's first AV.
                if av_prev is not None and av_prev[1] == H - 1:
                    emit_evac(t // H - 1)
                acc2 = acc2p.tile([128, 512], F32, tag="acc2")
                nc.vector.tensor_add(
                    out=acc2[:], in0=acc[:, 0, :], in1=acc[:, 1, :])
                racc = raccp.tile([128, 512], F32, tag="racc")
                nc.gpsimd.partition_all_reduce(
                    racc[:], acc2[:], 128, bass_isa.ReduceOp.add)
                rinv = rinvp.tile([128, 512], BF16, tag="rinv")
                with nc.allow_low_precision(reason="softmax scale recip"):
                    nc.vector.reciprocal(out=rinv[:], in_=racc[:])
                for e_t in es:
                    for half in range(2):
                        nc.vector.tensor_mul(
                            out=e_t[:, half, :], in0=e_t[:, half, :],
                            in1=rinv[:])
                return (es, h)

            emit_gbuild_half(0, 1)
            av_ctx = emit_stage(0, None)
            for h in range(1, H):
                emit_gbuild_half(h, 1)
            for t in range(1, IB * H):
                av_ctx = emit_stage(t, av_ctx)
            for jp in range(JP):
                emit_av_pair(av_ctx[0], av_ctx[1], jp)
            emit_evac(IB - 1)

    nc.compile()
    return nc


_NC_CACHE = {}


def _get_nc():
    if "v20" not in _NC_CACHE:
        _NC_CACHE["v20"] = build_nc_v20()
    return _NC_CACHE["v20"]


def _prep_in_maps(x, w_qkv, w_out, gamma, beta):
    x = np.ascontiguousarray(np.asarray(x), dtype=np.float32)
    w_qkv = np.asarray(w_qkv, dtype=np.float32)
    w_out = np.asarray(w_out, dtype=np.float32)
    gamma = np.asarray(gamma, dtype=np.float32)
    beta = np.asarray(beta, dtype=np.float32)
    assert x.shape == (N_CORES, N, DIM), x.shape
    if np.abs(beta).max() != 0.0:
        raise NotImplementedError("nonzero LayerNorm beta not supported")
    w_eff = w_qkv * gamma[None, :]
    M = np.concatenate([
        w_eff[H * DIM + h * DIM:H * DIM + (h + 1) * DIM, :].T @
        w_eff[h * DIM:(h + 1) * DIM, :]
        for h in range(H)
    ], axis=1)
    w_vo = np.concatenate([
        w_out[:, h * DIM:(h + 1) * DIM] @
        w_eff[2 * H * DIM + h * DIM:2 * H * DIM + (h + 1) * DIM, :]
        for h in range(H)
    ], axis=0)
    wqkvT = np.empty((DIM, 2 * H * DIM), np.float32)
    wqkvT[:, :H * DIM] = M
    wqkvT[:, H * DIM:] = w_vo.T
    wqkvT = np.ascontiguousarray(wqkvT).astype(np.float16)
    ident = np.eye(128, dtype=np.float32)
    return [
        {"x": np.ascontiguousarray(x[i]), "wqkvT": wqkvT, "ident": ident}
        for i in range(N_CORES)
    ]


def run(inputs, trace=False):
    nc = _get_nc()
    in_maps = _prep_in_maps(**inputs)
    res = run_bass_kernel_spmd(nc, in_maps, core_ids=list(range(N_CORES)),
                               trace=trace)
    out = np.stack([res.results[i]["out"] for i in range(N_CORES)], axis=0)
    return out, res


def kernel(**inputs) -> np.ndarray:
    out, _ = run(inputs, trace=False)
    return out


# revision 3
# speedup vs baseline: 1.5163x; 1.5163x over previous
"""Trainium2 Bass kernel for pre-LN multi-head self-attention.

One batch element per core (8 cores, data parallel). Host fuses weights:
M_h = Wk_h^T Wq_h (scores become xn M xn^T) and VO_h = W_o_h W_v_h (out-proj
folded into V). Per core:

  Phase 1 (interleaved): LN -> xnT fp16; V_h = xn VO_h^T -> vt[h] bf16 with a
    ones column (AV matmul then emits softmax row-sums for free); G^T_h =
    M_h^T xn^T -> gT[h] fp16 for ALL heads (the ib2=0 half rides inside the
    LN loop, the rest right after) so the attention phase never waits on G.
  Phase 2 (v21): flat (h, ib) block stream; per block 16x [scores matmul
    fp16 -> f32 PSUM; exp on scalar engine (constant -75 shift) -> e bf16].
    AV chunks flow through a GLOBAL pend queue of depth 5 that crosses block
    boundaries, so the tensor engine always has AV work to interleave with
    scores while exp latency drains -- no per-block pipeline drain. po
    accumulates [i, v | rowsum] in PSUM per (h, ib); its evacuation +
    normalize (reciprocal + tensor_scalar into y) run entirely on the DVE
    (keeping the scalar engine pure-exp in phase 2); out rows DMA as soon
    as the last head finishes them.

dtypes: fp16 scores side (bf16 logit noise costs 1.4e-2 rel err; fp16
2.4e-3), bf16 exp/V side (exp needs bf16 exponent range).
"""

import numpy as np

import concourse.bass as bass
import concourse.mybir as mybir
import concourse.tile as tile
from concourse import bacc
from concourse.bass_utils import run_bass_kernel_spmd

F32 = mybir.dt.float32
BF16 = mybir.dt.bfloat16
F16 = mybir.dt.float16

N_CORES = 8
N = 2048
DIM = 256
H = 8
EXP_SHIFT = 75.0

NT = N // 128
DC = DIM // 128
IB = N // 512
JP = NT // 2
VW = 258
PEND = 5


def build_nc_v21():
    nc = bacc.Bacc("TRN2", target_bir_lowering=False, debug=False,
                   num_devices=N_CORES)
    x_d = nc.dram_tensor("x", [N, DIM], F32, kind="ExternalInput")
    wq_d = nc.dram_tensor("wqkvT", [DIM, 2 * H * DIM], F16, kind="ExternalInput")
    id_d = nc.dram_tensor("ident", [128, 128], F32, kind="ExternalInput")
    out_d = nc.dram_tensor("out", [N, DIM], F32, kind="ExternalOutput")

    with tile.TileContext(nc) as tc:
        with (
            tc.tile_pool(name="singles", bufs=1) as singles,
            tc.tile_pool(name="xin", bufs=6) as xin,
            tc.tile_pool(name="lnst", bufs=6) as lnst,
            tc.tile_pool(name="etp", bufs=10) as etp,
            tc.tile_pool(name="tmpp", bufs=3) as tmpp,
            tc.tile_pool(name="pocp", bufs=2) as pocp,
            tc.tile_pool(name="rbp", bufs=4) as rbp,
            tc.tile_pool(name="ps_sc", bufs=2, space="PSUM") as ps_sc,
            tc.tile_pool(name="ps_acc", bufs=1, space="PSUM") as ps_acc,
        ):
            ident = singles.tile([128, 128], F32, tag="ident")
            nc.sync.dma_start(ident[:], id_d.ap()[:, :])
            eps_t = singles.tile([128, 1], F32, tag="eps")
            nc.vector.memset(eps_t, 1e-5)
            shift_t = singles.tile([128, 1], F32, tag="shift")
            nc.vector.memset(shift_t, -EXP_SHIFT)

            wqs = [[singles.tile([128, 2048], F16, tag=f"wq{dc}_{s}",
                                 name=f"wq{dc}_{s}") for s in range(2)]
                   for dc in range(DC)]
            xnT = [singles.tile([128, N], F16, tag=f"xnT{dc}", name=f"xnT{dc}")
                   for dc in range(DC)]
            vt = [singles.tile([128, NT, VW], BF16, tag=f"vt{h}", name=f"vt{h}")
                  for h in range(H)]
            gTa = [singles.tile([128, DC, N], F16, tag=f"gT{h}", name=f"gT{h}")
                   for h in range(H)]
            y = singles.tile([128, NT, DIM], F32, tag="y")

            def emit_ln(tcn):
                xt = xin.tile([128, DIM], F32, tag="xt")
                nc.sync.dma_start(xt[:], x_d.ap()[tcn * 128:(tcn + 1) * 128, :])
                stats = lnst.tile([128, 6], F32, tag="stats")
                nc.vector.bn_stats(out=stats[:], in_=xt[:])
                mv = lnst.tile([128, 2], F32, tag="mv")
                nc.vector.bn_aggr(out=mv[:], in_=stats[:])
                nc.scalar.activation(
                    out=mv[:, 1:2], in_=mv[:, 1:2],
                    func=mybir.ActivationFunctionType.Sqrt,
                    bias=eps_t[:, 0:1], scale=1.0)
                nc.vector.reciprocal(out=mv[:, 1:2], in_=mv[:, 1:2])
                # single-op normalize: the shortest cross-engine chain wins
                # over engine balance here (phase 1 is latency-bound).
                nc.vector.tensor_scalar(
                    out=xt[:], in0=xt[:], scalar1=mv[:, 0:1], scalar2=mv[:, 1:2],
                    op0=mybir.AluOpType.subtract, op1=mybir.AluOpType.mult)
                for dc in range(DC):
                    pst = ps_sc.tile([128, 2, 512], F32, tag="sc", name="pst")
                    nc.tensor.transpose(
                        pst[:, 0, :128], xt[:, dc * 128:(dc + 1) * 128], ident[:])
                    nc.vector.tensor_copy(
                        out=xnT[dc][:, tcn * 128:(tcn + 1) * 128],
                        in_=pst[:, 0, :128])

            vcopy_idx = [0]

            def emit_vbuild(tc2):
                # V-build PSUMs live in the acc pool (its 4 banks are idle
                # until attention starts), taking pressure off the 2 "sc"
                # slots that pace the LN-transpose/G-build rotation.
                for g in range(2):  # head-pair groups (2 pairs per acc tile)
                    ps = ps_acc.tile([128, 4, 512], F32, tag="acc", name="psv")
                    for pi in range(2):
                        p = 2 * g + pi
                        for half in range(2):
                            tcn = tc2 * 2 + half
                            for dc in range(DC):
                                nc.tensor.matmul(
                                    ps[:, 2 * pi + half, :],
                                    xnT[dc][:, tcn * 128:(tcn + 1) * 128],
                                    wqs[dc][1][:, p * 512:(p + 1) * 512],
                                    start=(dc == 0), stop=(dc == DC - 1))
                        for s in range(2):
                            # V copies split ~2:1 scalar:vector so neither
                            # queue paces phase 1.
                            dst = vt[2 * p + s][:, tc2 * 2:tc2 * 2 + 2, 0:256]
                            src = ps[:, 2 * pi:2 * pi + 2,
                                     s * 256:(s + 1) * 256]
                            if vcopy_idx[0] % 3 != 2:
                                nc.scalar.copy(out=dst, in_=src)
                            else:
                                nc.vector.tensor_copy(out=dst, in_=src)
                            vcopy_idx[0] += 1

            def emit_gbuild_half(h, ib2):
                gT = gTa[h]
                for mc in range(DC):
                    ps = ps_sc.tile([128, 2, 512], F32, tag="sc", name="psg")
                    for half in range(2):
                        jb = ib2 * 2 + half
                        for dc in range(DC):
                            nc.tensor.matmul(
                                ps[:, half, :],
                                wqs[dc][0][:, h * DIM + mc * 128:
                                           h * DIM + (mc + 1) * 128],
                                xnT[dc][:, jb * 512:(jb + 1) * 512],
                                start=(dc == 0), stop=(dc == DC - 1))
                    dst = gT[:, mc, ib2 * 1024:(ib2 + 1) * 1024]
                    if (h + mc) % 2 == 0:
                        nc.scalar.copy(out=dst, in_=ps[:, :, :])
                    else:
                        nc.vector.tensor_copy(out=dst, in_=ps[:, :, :])

            for h in range(H):
                nc.vector.memset(vt[h][:, :, 256:257], 1.0)

            emit_ln(0)
            emit_ln(1)
            for dc in range(DC):  # VO weights behind x0/x1 (V build first)
                nc.sync.dma_start(
                    wqs[dc][1][:],
                    wq_d.ap()[dc * 128:(dc + 1) * 128, 2048:4096])
            for tc2 in range(1, NT // 2):
                emit_ln(tc2 * 2)
                if tc2 == 1:
                    for dc in range(DC):  # M weights behind x2
                        nc.sync.dma_start(
                            wqs[dc][0][:],
                            wq_d.ap()[dc * 128:(dc + 1) * 128, 0:2048])
                emit_ln(tc2 * 2 + 1)
                emit_vbuild(tc2 - 1)
                if tc2 >= 4:  # G first halves once token chunks 0..7 exist
                    emit_gbuild_half(2 * (tc2 - 4), 0)
                    emit_gbuild_half(2 * (tc2 - 4) + 1, 0)
            emit_vbuild(NT // 2 - 1)
            for h in range(H):
                emit_gbuild_half(h, 1)

            # ---- Phase 2: flat block stream with a global AV pend queue ----
            po_cur = [None]
            pend = []

            def emit_finish(h, ib):
                # po evacuation + normalize, entirely on the DVE: the scalar
                # engine stays pure-exp so the next block's exps are never
                # delayed behind copies.
                po = po_cur[0]
                poc = pocp.tile([128, 4, 257], F32, tag="poc")
                nc.vector.tensor_copy(out=poc[:, 0:2, :], in_=po[:, 0:2, 0:257])
                nc.vector.tensor_copy(out=poc[:, 2:4, :], in_=po[:, 2:4, 0:257])
                rb = rbp.tile([128, 4, 1], F32, tag="rb")
                nc.vector.reciprocal(out=rb[:, :, :], in_=poc[:, :, 256:257])
                for k in range(4):
                    i128 = ib * 4 + k
                    if h == 0:
                        nc.vector.tensor_scalar(
                            out=y[:, i128, :], in0=poc[:, k, 0:256],
                            scalar1=rb[:, k, 0:1], scalar2=None,
                            op0=mybir.AluOpType.mult)
                    else:
                        tmpt = tmpp.tile([128, DIM], F32, tag="tmp")
                        nc.vector.tensor_scalar(
                            out=tmpt[:], in0=poc[:, k, 0:256],
                            scalar1=rb[:, k, 0:1], scalar2=None,
                            op0=mybir.AluOpType.mult)
                        nc.vector.tensor_add(
                            out=y[:, i128, :], in0=y[:, i128, :],
                            in1=tmpt[:])
                    if h == H - 1:
                        nc.sync.dma_start(
                            out_d.ap()[i128 * 128:(i128 + 1) * 128, :],
                            y[:, i128, :])

            def emit_one():
                h, ib, jp, e_t = pend.pop(0)
                if jp == 0:
                    po_cur[0] = ps_acc.tile([128, 4, 512], F32, tag="acc",
                                            name="po")
                po = po_cur[0]
                vth = vt[h]
                for half in range(2):
                    jc = jp * 2 + half
                    for k in range(4):
                        nc.tensor.matmul(
                            po[:, k, 0:257],
                            e_t[:, half, k * 128:(k + 1) * 128],
                            vth[:, jc, 0:257],
                            start=(jp == 0 and half == 0), stop=(jc == NT - 1))
                if jp == JP - 1:
                    emit_finish(h, ib)

            for h in range(H):
                for ib in range(IB):
                    last = (h == H - 1 and ib == IB - 1)
                    for jp in range(JP):
                        ps = ps_sc.tile([128, 2, 512], F32, tag="sc",
                                        name="pss")
                        for half in range(2):
                            jc = jp * 2 + half
                            for dc in range(DC):
                                nc.tensor.matmul(
                                    ps[:, half, :],
                                    gTa[h][:, dc, jc * 128:(jc + 1) * 128],
                                    xnT[dc][:, ib * 512:(ib + 1) * 512],
                                    start=(dc == 0), stop=(dc == DC - 1))
                        e_t = etp.tile([128, 2, 512], BF16, tag="et")
                        nc.scalar.activation(
                            out=e_t[:, :, :], in_=ps[:, :, :],
                            func=mybir.ActivationFunctionType.Exp,
                            bias=shift_t[:, 0:1], scale=1.0)
                        pend.append((h, ib, jp, e_t))
                        # the last block drains eagerly so the kernel tail is
                        # just one AV chunk + finish, not a full pend flush.
                        depth = 1 if last else PEND
                        while len(pend) > depth:
                            emit_one()
            while pend:
                emit_one()

    nc.compile()
    return nc


_NC_CACHE = {}


def _get_nc():
    if "v21" not in _NC_CACHE:
        _NC_CACHE["v21"] = build_nc_v21()
    return _NC_CACHE["v21"]


def _prep_in_maps(x, w_qkv, w_out, gamma, beta):
    x = np.ascontiguousarray(np.asarray(x), dtype=np.float32)
    w_qkv = np.asarray(w_qkv, dtype=np.float32)
    w_out = np.asarray(w_out, dtype=np.float32)
    gamma = np.asarray(gamma, dtype=np.float32)
    beta = np.asarray(beta, dtype=np.float32)
    assert x.shape == (N_CORES, N, DIM), x.shape
    if np.abs(beta).max() != 0.0:
        raise NotImplementedError("nonzero LayerNorm beta not supported")
    w_eff = w_qkv * gamma[None, :]
    M = np.concatenate([
        w_eff[H * DIM + h * DIM:H * DIM + (h + 1) * DIM, :].T @
        w_eff[h * DIM:(h + 1) * DIM, :]
        for h in range(H)
    ], axis=1)
    w_vo = np.concatenate([
        w_out[:, h * DIM:(h + 1) * DIM] @
        w_eff[2 * H * DIM + h * DIM:2 * H * DIM + (h + 1) * DIM, :]
        for h in range(H)
    ], axis=0)
    wqkvT = np.empty((DIM, 2 * H * DIM), np.float32)
    wqkvT[:, :H * DIM] = M
    wqkvT[:, H * DIM:] = w_vo.T
    wqkvT = np.ascontiguousarray(wqkvT).astype(np.float16)
    ident = np.eye(128, dtype=np.float32)
    return [
        {"x": np.ascontiguousarray(x[i]), "wqkvT": wqkvT, "ident": ident}
        for i in range(N_CORES)
    ]


def run(inputs, trace=False):
    nc = _get_nc()
    in_maps = _prep_in_maps(**inputs)
    res = run_bass_kernel_spmd(nc, in_maps, core_ids=list(range(N_CORES)),
                               trace=trace)
    out = np.stack([res.results[i]["out"] for i in range(N_CORES)], axis=0)
    return out, res


def kernel(**inputs) -> np.ndarray:
    out, _ = run(inputs, trace=False)
    return out
